# revision 4
# baseline (speedup 1.0000x reference)
"""Trainium2 Bass kernel for GsumLayer dense branch: out[b] = a[b] @ x[b].

Shapes (hardcoded): B=8, N=4096, D=32, fp32 in/out.
Sharding: one batch element per NeuronCore (8 cores, data parallel).

fp8 strategy (memory-bound; ~16MB of A per core instead of bf16's 32MB):
  - Host quantizes A' = (a[b] - 0.5) to fp8 e4m3 and transposes -> aT8 [k, n].
    The 0.5 shift halves quantization error (|A'| <= 0.5); the exact rank-1
    correction 0.5*colsum_fp32(x) is added back on the host.
  - x is split into two e4m3 columns x_hi = q(x), x_lo = q(x - x_hi) so the
    x-side quantization error is negligible; both stream against the same A'
    (stationary [128, 2, 64] = [x_hi | x_lo]).
  - perf_mode=DoubleRow packs 2 fp8 weights/cell: each matmul contracts
    K=256 (pair dim) and streams 2 A-bytes/row/cycle -> ~2x PE throughput
    and half the HBM bytes of the bf16 baseline.
  - A DMAs use a pair-plane split: each queue reads 128 consecutive 4KB rows
    (fully contiguous 512KB source block) into all 128 partitions.
  - PSUM [64, 4096] fp32: partitions 0-31 = (A'@x_hi)^T, 32-63 = (A'@x_lo)^T.
    ACT stages L cross-quadrant into SBUF, DVE adds H (psum) + L (sbuf);
    chunks DMA out as their accumulation completes.
  - Host: out[b] = (H+L).T + bias.  Measured rel err 1.19e-2 (tol 2e-2).

Measured (For_i hardware-loop differential, K=128 vs 1024, min over reps):
  58.8 us/core/iteration vs 105 us for the bf16 baseline by the same method
  (1.79x).  DMA-bound: 16.25MB/iter at ~330 GB/s effective (of ~358 GB/s
  HBM-per-NC); pure-DMA floor measured 50.4 us, PE (DoubleRow) ~36 us.
"""

import numpy as np
import ml_dtypes

B, N, D = 8, 4096, 32
P = 128
KT = N // (2 * P)     # 16 k-super-tiles of 256 rows (DoubleRow pair)
FREE = 512            # matmul free dim (one PSUM bank of f32)
NCH = N // FREE       # 8 n-chunks

_cache = {}


def _build(iters=None):
    """Single-shot kernel when iters is None; otherwise the same body wrapped
    in an in-NEFF For_i loop (used by the local bench harness only)."""
    import contextlib

    import concourse.bass as bass
    import concourse.mybir as mybir
    import concourse.tile as tile
    from concourse import bacc

    f32 = mybir.dt.float32
    fp8 = mybir.dt.float8e4
    DR = mybir.MatmulPerfMode.DoubleRow

    nc = bacc.Bacc("TRN2", target_bir_lowering=False, debug=False)
    x_d = nc.dram_tensor("x", [N, 2 * D], fp8, kind="ExternalInput")   # [k, 64]
    a_d = nc.dram_tensor("at", [N, N], fp8, kind="ExternalInput")      # A'^T [k, n]
    o_d = nc.dram_tensor("ct", [D, N], f32, kind="ExternalOutput")     # (H+L) [d, n]

    with tile.TileContext(nc) as tc:
        with (
            tc.tile_pool(name="xp", bufs=1) as xpool,
            tc.tile_pool(name="atb", bufs=5) as atpool,
            tc.tile_pool(name="cout", bufs=2) as copool,
            tc.tile_pool(name="psc", bufs=1, space=bass.MemorySpace.PSUM) as psc,
        ):
            # stage the x load: the 16KB slice kt=0 needs comes first so the
            # scalar queue reaches kt0's A-plane almost immediately; the rest
            # follows (needed only ~3us later, before kt1's matmuls).
            x_sb = xpool.tile([P, KT, 2, 2 * D], fp8)
            nc.scalar.dma_start(
                x_sb[:, 0],
                x_d[0 : 2 * P, :].rearrange("(i p) m -> p i m", i=2, p=P),
            )
            nc.scalar.dma_start(
                x_sb[:, 1:],
                x_d[2 * P :, :].rearrange(
                    "(kt i p) m -> p kt i m", kt=KT - 1, i=2, p=P
                ),
            )

            loop = tc.For_i(0, iters) if iters is not None else contextlib.nullcontext()
            with loop:
                c_sb = copool.tile([D, N], f32)
                l_sb = copool.tile([D, N], f32)
                ct = psc.tile([2 * D, N], f32)

                for kt in range(KT):
                    aT = atpool.tile([P, 2, N], fp8)
                    base = kt * 2 * P
                    if kt == 0:
                        # quarter-split so chunk-0 matmuls start ~1us sooner
                        q = N // 4
                        nc.sync.dma_start(aT[:, 0, :q], a_d[base : base + P, :q])
                        nc.scalar.dma_start(
                            aT[:, 1, :q], a_d[base + P : base + 2 * P, :q]
                        )
                        nc.sync.dma_start(aT[:, 0, q:], a_d[base : base + P, q:])
                        nc.scalar.dma_start(
                            aT[:, 1, q:], a_d[base + P : base + 2 * P, q:]
                        )
                    else:
                        nc.sync.dma_start(aT[:, 0], a_d[base : base + P, :])
                        nc.scalar.dma_start(aT[:, 1], a_d[base + P : base + 2 * P, :])
                    for c in range(NCH):
                        sl = slice(c * FREE, (c + 1) * FREE)
                        nc.tensor.matmul(
                            ct[:, sl],
                            x_sb[:, kt],
                            aT[:, :, sl],
                            start=(kt == 0),
                            stop=(kt == KT - 1),
                            perf_mode=DR,
                        )
                for c in range(NCH):
                    sl = slice(c * FREE, (c + 1) * FREE)
                    # DVE can't read two PSUM operands in one op (single PSUM
                    # port): ACT stages L (psum parts 32-63 -> sbuf parts
                    # 0-31), then DVE adds H (psum) + L (sbuf).
                    nc.scalar.copy(l_sb[:, sl], ct[D : 2 * D, sl])
                    nc.vector.tensor_add(c_sb[:, sl], ct[0:D, sl], l_sb[:, sl])
                    eng = nc.sync if c % 2 == 0 else nc.scalar
                    eng.dma_start(o_d[:, sl], c_sb[:, sl])

    nc.compile()
    return nc


FP8 = ml_dtypes.float8_e4m3fn


def _prep(x_b: np.ndarray, a_b: np.ndarray):
    """Host-side quantization for one batch element."""
    xh = x_b.astype(FP8)
    xl = (x_b - xh.astype(np.float32)).astype(FP8)
    x64 = np.concatenate([xh, xl], axis=1)  # [N, 64] fp8
    at8 = np.ascontiguousarray((a_b - 0.5).astype(FP8).T)  # [k, n] fp8
    return {"x": x64, "at": at8}


def kernel(x: np.ndarray, a: np.ndarray) -> np.ndarray:
    from concourse.bass_utils import run_bass_kernel_spmd

    x = np.asarray(x, dtype=np.float32)
    a = np.asarray(a, dtype=np.float32)
    assert x.shape == (B, N, D) and a.shape == (B, N, N)

    if "nc" not in _cache:
        _cache["nc"] = _build()

    in_maps = [_prep(x[b], a[b]) for b in range(B)]
    res = run_bass_kernel_spmd(_cache["nc"], in_maps, core_ids=list(range(B)))
    hl = np.stack([r["ct"] for r in res.results])  # [B, D, N] fp32 = H + L
    bias = 0.5 * x.sum(axis=1)  # [B, D] exact fp32 colsum correction
    out = hl.transpose(0, 2, 1) + bias[:, None, :]
    return np.ascontiguousarray(out).astype(np.float32)
